# revision 5
# baseline (speedup 1.0000x reference)
"""Trainium2 Bass kernel for masked-softmax attention (sparse_attention).

reference:
    S = Q @ K^T / sqrt(128)            # [N, nq, nk]
    A = softmax(S, axis=-1) * mask
    A = A / (sum_k A + 1e-6)
    O = A @ V

Device identity (softmax normalizer cancels in the renormalization):
    E = exp(S); P = E * mask
    O[q, :] = (P @ V)[q, :] / sum_k P[q, k]
(the reference's +1e-6 is ~2e-6 relative to the masked sum and dropped).

Sharding: N=32 batch-heads split across 8 NeuronCores, 4 per core; no
cross-core communication. Host staging is layout/dtype only: Q/K
transposed to [d, n] bf16, V tiled with a fused ones-column (gives the
renormalization denominator as a 129th matmul output column), mask
transposed to [k, q] bf16 and laid out so each (batch, q-half, k-half)
block is contiguous (16KB/partition runs -> minimal DMA descriptors).

Per-core pipeline over 8 slabs (batch b, q-half h of 1024):
  slab prologue: prefetch NEXT slab's mask (2 big DMAs, lo/hi k-half
    tiles) and, at h==0, next batch's K^T/Q^T/V, so DMA runs a slab ahead.
  k-phase, per k-tile kt (128 rows of K):
    mm1  (PE, bf16): ST_kt = KT_kt.T @ QT_h   [128k x 1024q] -> PSUM
    exp  (ACT):      ET = exp(ST/sqrt(d)) [bf16]             -> SBUF
    mult (DVE, 2x):  PT[kt] = ET * maskT_kt [bf16]           -> P^T slab
  q-phase (interleaved one q-tile per 2 kt, on the PREVIOUS slab's
  finished P^T):
    mm2  (PE, bf16): O|denom = sum_kt PT[kt][:,qc].T @ [V_kt | 1] -> PSUM
    recip+scale (DVE): st[qc] = O * (1/denom)  [bf16]
  store st -> out per slab (8 q-tiles, sync HWDGE ring).
"""
import sys

sys.path.insert(0, "/opt/trn_rl_repo")

import ml_dtypes
import numpy as np

from concourse import bacc, mybir, tile
from concourse.bass_utils import run_bass_kernel_spmd

N, NQ, NK, D = 32, 2048, 2048, 128
N_CORES = 8
B = N // N_CORES          # batches per core
KT = NK // 128            # k tiles per batch
QT = NQ // 128            # q tiles per batch
QTH = QT // 2             # q tiles per slab
QH = NQ // 2              # q-half width
KH = KT // 2              # k tiles per mask half-tile
SCALE = float(1.0 / np.sqrt(D))

F32 = mybir.dt.float32
BF16 = mybir.dt.bfloat16

_cached = {}


def build():
    if "nc" in _cached:
        return _cached["nc"]
    nc = bacc.Bacc("TRN2", target_bir_lowering=False, debug=False)

    qt_d = nc.dram_tensor("queriesT", [B, 2, D, QH], BF16, kind="ExternalInput").ap()
    kt_d = nc.dram_tensor("keysT", [B, D, NK], BF16, kind="ExternalInput").ap()
    v_d = nc.dram_tensor("valuesP", [B, 128, KT, D + 1], BF16, kind="ExternalInput").ap()
    m_d = nc.dram_tensor("maskT", [B, 2, 2, 128, KH, QH], BF16, kind="ExternalInput").ap()
    o_d = nc.dram_tensor("out", [B, 2, 128, QTH, D], BF16, kind="ExternalOutput").ap()

    with tile.TileContext(nc) as tc:
        with (
            tc.tile_pool(name="tr", bufs=2) as trpool,
            tc.tile_pool(name="qth", bufs=4) as qpool,
            tc.tile_pool(name="vbo", bufs=2) as vpool,
            tc.tile_pool(name="maskt", bufs=2) as mpool,
            tc.tile_pool(name="work", bufs=4) as wpool,
            tc.tile_pool(name="ptslab", bufs=2) as ptpool,
            tc.tile_pool(name="stage", bufs=4) as stpool,
            tc.tile_pool(name="spsum", bufs=2, space="PSUM") as spool,
            tc.tile_pool(name="opsum", bufs=4, space="PSUM") as opool,
        ):
            def issue_mask(i):
                b, h = divmod(i, 2)
                mlo = mpool.tile([128, KH, QH], BF16, tag="mlo")
                mhi = mpool.tile([128, KH, QH], BF16, tag="mhi")
                nc.sync.dma_start(mlo[:], m_d[b, h, 0])
                nc.sync.dma_start(mhi[:], m_d[b, h, 1])
                return mlo, mhi

            def issue_batch(bb):
                kt_a = trpool.tile([128, 512], BF16, tag="kta")
                kt_b = trpool.tile([128, NK - 512], BF16, tag="ktb")
                qt_h0 = qpool.tile([128, QH], BF16, tag="qt")
                qt_h1 = qpool.tile([128, QH], BF16, tag="qt")
                vb = vpool.tile([128, KT, D + 1], BF16, tag="vb")
                nc.sync.dma_start(kt_a[:], kt_d[bb, :, 0:512])
                nc.sync.dma_start(qt_h0[:], qt_d[bb, 0])
                nc.sync.dma_start(kt_b[:], kt_d[bb, :, 512:NK])
                nc.sync.dma_start(qt_h1[:], qt_d[bb, 1])
                nc.sync.dma_start(vb[:], v_d[bb])
                return kt_a, kt_b, (qt_h0, qt_h1), vb

            def issue_batch0():
                # batch 0: ordered so mm1(kt=0) and the first mask half
                # arrive as early as possible
                kt_a = trpool.tile([128, 512], BF16, tag="kta")
                kt_b = trpool.tile([128, NK - 512], BF16, tag="ktb")
                qt_h0 = qpool.tile([128, QH], BF16, tag="qt")
                qt_h1 = qpool.tile([128, QH], BF16, tag="qt")
                vb = vpool.tile([128, KT, D + 1], BF16, tag="vb")
                mlo = mpool.tile([128, KH, QH], BF16, tag="mlo")
                mhi = mpool.tile([128, KH, QH], BF16, tag="mhi")
                nc.sync.dma_start(kt_a[:], kt_d[0, :, 0:512])
                nc.sync.dma_start(qt_h0[:], qt_d[0, 0])
                nc.sync.dma_start(mlo[:], m_d[0, 0, 0])
                nc.sync.dma_start(kt_b[:], kt_d[0, :, 512:NK])
                nc.sync.dma_start(mhi[:], m_d[0, 0, 1])
                nc.sync.dma_start(qt_h1[:], qt_d[0, 1])
                nc.sync.dma_start(vb[:], v_d[0])
                return kt_a, kt_b, (qt_h0, qt_h1), vb, (mlo, mhi)

            def q_iter(prev, qc):
                """One q-tile of the q-phase for a finished P^T slab."""
                pt, vb, st, b, h = prev
                o_ps = opool.tile([128, D + 1], F32, tag="o")
                for kt in range(KT):
                    nc.tensor.matmul(
                        o_ps[:],
                        pt[:, kt, qc * 128:(qc + 1) * 128],
                        vb[:, kt, :],
                        start=(kt == 0),
                        stop=(kt == KT - 1),
                    )
                rd = wpool.tile([128, 1], F32, tag="rd")
                nc.vector.reciprocal(rd[:], o_ps[:, D:D + 1])
                nc.vector.tensor_scalar_mul(st[:, qc, :], o_ps[:, 0:D], rd[:])
                if qc == QTH - 1:
                    nc.sync.dma_start(o_d[b, h], st[:])

            kt_a, kt_b, qt_hs, vb, mask_cur = issue_batch0()

            prev = None
            for i in range(2 * B):
                b, h = divmod(i, 2)
                # prefetch next slab's mask, then next batch's K/Q/V
                mask_next = issue_mask(i + 1) if i + 1 < 2 * B else None
                if h == 0 and b + 1 < B:
                    nxt = issue_batch(b + 1)
                st = stpool.tile([128, QTH, D], BF16, tag="st")

                qt_sb = qt_hs[h]
                pt = ptpool.tile([128, KT, QH], BF16, tag="pt")
                for kt in range(KT):
                    s_ps = spool.tile([128, QH], F32, tag="s")
                    ksrc = kt_a[:, kt * 128:(kt + 1) * 128] if kt < 4 else \
                        kt_b[:, (kt - 4) * 128:(kt - 3) * 128]
                    for c in range(2):
                        nc.tensor.matmul(
                            s_ps[:, c * 512:(c + 1) * 512],
                            ksrc,
                            qt_sb[:, c * 512:(c + 1) * 512],
                            start=True,
                            stop=True,
                        )
                    e_sb = wpool.tile([128, QH], BF16, tag="e")
                    nc.scalar.activation(
                        e_sb[:],
                        s_ps[:],
                        mybir.ActivationFunctionType.Exp,
                        scale=SCALE,
                    )
                    nc.vector.tensor_tensor(
                        out=pt[:, kt, :],
                        in0=e_sb[:],
                        in1=mask_cur[kt // KH][:, kt % KH, :],
                        op=mybir.AluOpType.mult,
                    )
                    # interleave the previous slab's q-phase into this
                    # k-phase (one q-tile per two k-tiles)
                    if prev is not None and kt % 2 == 0:
                        q_iter(prev, kt // 2)
                prev = (pt, vb, st, b, h)
                mask_cur = mask_next
                if h == 1 and b + 1 < B:
                    kt_a, kt_b, qt_hs, vb = nxt

            for qc in range(QTH):
                q_iter(prev, qc)

    nc.compile()
    _cached["nc"] = nc
    return nc


def kernel(queries, keys, values, mask, _trace=False, **kw):
    queries = np.asarray(queries, dtype=np.float32)
    keys = np.asarray(keys, dtype=np.float32)
    values = np.asarray(values, dtype=np.float32)
    mask = np.asarray(mask, dtype=np.float32)
    nc = build()
    bf16 = ml_dtypes.bfloat16
    in_maps = []
    for c in range(N_CORES):
        sl = slice(c * B, (c + 1) * B)
        vv = values[sl].reshape(B, KT, 128, D).transpose(0, 2, 1, 3)
        v_aug = np.concatenate(
            [vv, np.ones((B, 128, KT, 1), np.float32)], axis=3
        )
        in_maps.append(
            {
                "queriesT": np.ascontiguousarray(
                    queries[sl].transpose(0, 2, 1).reshape(B, D, 2, QH)
                    .transpose(0, 2, 1, 3)
                ).astype(bf16),
                "keysT": np.ascontiguousarray(
                    keys[sl].transpose(0, 2, 1)
                ).astype(bf16),
                "valuesP": np.ascontiguousarray(v_aug).astype(bf16),
                "maskT": np.ascontiguousarray(
                    mask[sl]
                    .transpose(0, 2, 1)
                    .reshape(B, 2, KH, 128, 2, QH)
                    .transpose(0, 4, 1, 3, 2, 5)
                ).astype(bf16),
            }
        )
    res = run_bass_kernel_spmd(
        nc, in_maps, core_ids=list(range(N_CORES)), trace=_trace
    )
    out = np.concatenate(
        [
            res.results[c]["out"]
            .astype(np.float32)
            .transpose(0, 1, 3, 2, 4)
            .reshape(B, NQ, D)
            for c in range(N_CORES)
        ],
        axis=0,
    )
    if _trace:
        return out, res
    return out
